# revision 18
# baseline (speedup 1.0000x reference)
"""DTW frames layer on 8 Trainium2 NeuronCores.

Reference computation (per (n, k) problem):
    cost[p, w] = max(0, ||x[n, :, w] - patts[k, :, p]||^2)          (P=32, W=128)
    dtw[0, w]  = cumsum_w cost[0, w]
    dtw[p, 0]  = cumsum_p cost[p, 0]
    dtw[p, w]  = cost[p, w] + min(dtw[p, w-1], dtw[p-1, w-1], dtw[p-1, w])
    out        = sqrt(dtw[:, -32:]) / 32

Strategy:
  - Data-parallel over batch n: each of the 8 cores owns n_loc = 8 rows of x,
    patterns replicated.
  - Cost matrix via one augmented K=10 matmul per (M-chunk, n-chunk):
    lhsT rows = [-2*patts[d], ||patt||^2, 1], rhs rows = [x[d], 1, ||x||^2],
    so PSUM = cost before clamping; ReLU on eviction applies the max(0, .).
  - Matmul emits [(k4 p32), (n4 w)] tiles; a strided SBUF->SBUF DMA permutes
    into scan layout C[(n4 k32), p, w] (partition = independent problem).
  - DTW row recurrence runs on the DVE tensor_tensor_scan instruction:
    state = (m[t] min state) add c[t], one instruction per table row, where
    m[t] = min(dtw[p-1, t-1], dtw[p-1, t]) is one shifted tensor_tensor min.
    Column 0 of the m operand stays at +BIG so element 0 of each scan reduces
    to initial + cost (the first-column cumsum), with initial = prev row's
    column 0.
  - sqrt(dtw)/32 = Sqrt activation with scale 1/1024 on the last 32 columns.
"""

import numpy as np

import concourse.bass as bass
import concourse.mybir as mybir
import concourse.tile as tile
from concourse.bass_utils import run_bass_kernel_spmd

N, D, W = 64, 8, 128      # x: (N, D, W)
K, P = 32, 32             # patts: (K, D, P)
WO = 32                   # output keeps last WO columns of the DTW table
NCORES = 8
NLOC = N // NCORES        # 8 batch rows per core
NT = 2                    # problem tiles per core: (4 n x 32 k) = 128 partitions
KAUG = D + 2              # augmented contraction dim
BIG = 1e30

f32 = mybir.dt.float32


def build_program() -> bass.Bass:
    nc = bass.Bass()
    # lhs (10, 1024) and rhs (10, 1024) packed side by side so a single DMA
    # (one queue, one semaphore) loads both: PE Matmult only has one ISA
    # sync-wait slot.
    inp_d = nc.dram_tensor("inp", (KAUG, K * P + NLOC * W), f32, kind="ExternalInput")
    out_d = nc.dram_tensor("out", (NLOC, K, P, WO), f32, kind="ExternalOutput")

    from concourse.tile import add_dep_helper

    with tile.TileContext(nc) as tc:
        with (
            tc.tile_pool(name="consts", bufs=1) as consts,
            tc.tile_pool(name="psum", bufs=4, space="PSUM") as psum_pool,
            tc.tile_pool(name="mm", bufs=16) as mm_pool,
            tc.tile_pool(name="cbuf", bufs=2) as c_pool,
            tc.tile_pool(name="vbuf", bufs=64) as v_pool,
            tc.tile_pool(name="mbuf", bufs=2) as m_pool,
            tc.tile_pool(name="obuf", bufs=2) as o_pool,
        ):
            inp_s = consts.tile([KAUG, K * P + NLOC * W], f32)
            nc.sync.dma_start(out=inp_s, in_=inp_d[:, :])
            lhs_s = inp_s[:, 0:K * P]
            rhs_s = inp_s[:, K * P:K * P + NLOC * W]
            # Fence scratch: 1-element tiles written by "wait absorber" ops.
            # Every ISA instruction has ONE sync-wait slot; a DMA triggered
            # right after a compute op would need two waits (producer +
            # queue-predecessor). A tiny same-engine op that reads the
            # producer's output absorbs the producer wait, and the engine
            # sequencer's program order then covers it for the DMAs.
            facta = consts.tile([1, 1], f32)
            factd = [
                consts.tile([32, 1], f32, name=f"factd{i}", tag=f"factd{i}")
                for i in range(4 * NT)
            ]

            # Cost tiles in scan layout: partition = (n4 k32), free = (p, w).
            C = [
                c_pool.tile([128, P, W], f32, tag=f"C{t}", name=f"C{t}")
                for t in range(NT)
            ]

            last_mm = None
            last_perm = {}               # (t, nn) -> last permute DMA handle
            for m in range(8):           # M-chunk: k in [4m, 4m+4), all p
                for t in range(NT):      # n-chunk: n in [4t, 4t+4)
                    ps = psum_pool.tile([128, 512], f32)
                    last_mm = nc.tensor.matmul(
                        ps,
                        lhs_s[:, m * 128:(m + 1) * 128],
                        rhs_s[:, t * 512:(t + 1) * 512],
                        start=True,
                        stop=True,
                    )
                    mm = mm_pool.tile([128, 512], f32)
                    nc.scalar.activation(mm, ps, mybir.ActivationFunctionType.Relu)
                    fence = nc.scalar.activation(
                        facta, mm[0:1, 0:1], mybir.ActivationFunctionType.Copy
                    )
                    for nn in range(4):
                        base = nn * 32 + 4 * m
                        dma = nc.scalar.dma_start(
                            out=C[t][base:base + 4, :, :],
                            in_=mm[:, nn * 128:(nn + 1) * 128],
                        )
                        add_dep_helper(
                            dma.ins, fence.ins, sync=False,
                            reason="permute DMA after ACT wait-absorber",
                        )
                        last_perm[(t, nn)] = dma

            ot_tiles = []
            last_scan = []
            last_ofence = None
            odmas = []
            for t in range(NT):
                mt = m_pool.tile([128, W], f32)
                nc.vector.memset(mt, BIG)
                ot = o_pool.tile([128, P, WO], f32)
                # DVE wait absorbers: one per HWDGE queue feeding C[t]; the
                # last permute on each queue covers the 7 before it (FIFO).
                dve_fences = []
                for nn in range(4):
                    df = nc.vector.tensor_copy(
                        factd[t * 4 + nn], C[t][nn * 32:(nn + 1) * 32, 0, 0:1]
                    )
                    dve_fences.append(df)
                vprev = None
                scan = None
                for p in range(P):
                    v = v_pool.tile([128, W], f32)
                    if p == 0:
                        scan = nc.vector.tensor_tensor_scan(
                            v, mt, C[t][:, 0, :], 0.0,
                            mybir.AluOpType.min, mybir.AluOpType.add,
                        )
                        for df in dve_fences:
                            add_dep_helper(
                                scan.ins, df.ins, sync=False,
                                reason="first scan after DVE wait-absorbers",
                            )
                    else:
                        nc.vector.tensor_tensor(
                            mt[:, 1:W], vprev[:, 0:W - 1], vprev[:, 1:W],
                            mybir.AluOpType.min,
                        )
                        scan = nc.vector.tensor_tensor_scan(
                            v, mt, C[t][:, p, :], vprev[:, 0:1],
                            mybir.AluOpType.min, mybir.AluOpType.add,
                        )
                    nc.scalar.activation(
                        ot[:, p, :], v[:, W - WO:W],
                        mybir.ActivationFunctionType.Sqrt, scale=1.0 / (P * P),
                    )
                    vprev = v
                last_scan.append(scan)
                ofence = nc.scalar.activation(
                    facta, ot[0:1, P - 1, 0:1], mybir.ActivationFunctionType.Copy
                )
                last_ofence = ofence
                odma = nc.scalar.dma_start(
                    out=out_d[t * 4:(t + 1) * 4, :, :, :], in_=ot
                )
                add_dep_helper(
                    odma.ins, ofence.ins, sync=False,
                    reason="out DMA after ACT wait-absorber",
                )
                ot_tiles.append(ot)
                odmas.append(odma)

            # Feed every proc's final tick into the sync sequencer via a nop
            # chain (one 1-slot wait each) so the kernel-tail drain's wait
            # list elides to nothing: per-proc wait elision is direct, not
            # transitive, and the drain nop also has a single ISA wait slot.
            tail_deps = []
            for t in range(NT):
                for nn in range(4):
                    tail_deps.append(last_perm[(t, nn)])
            tail_deps += odmas + [last_ofence, last_mm] + last_scan
            prev_nop = None
            for td in tail_deps:
                nop = nc.sync.nop()
                add_dep_helper(
                    nop.ins, td.ins, sync=True,
                    reason="drain pre-absorber: sync waits on proc tail",
                )
                if prev_nop is not None:
                    add_dep_helper(
                        nop.ins, prev_nop.ins, sync=False,
                        reason="keep nop chain ordered",
                    )
                prev_nop = nop
    return nc


def make_in_maps(x: np.ndarray, patts: np.ndarray) -> list[dict[str, np.ndarray]]:
    x = np.ascontiguousarray(x, dtype=np.float32)
    patts = np.ascontiguousarray(patts, dtype=np.float32)
    pf = patts.transpose(1, 0, 2).reshape(D, K * P)              # [d, (k p)]
    p2 = (patts * patts).sum(axis=1).reshape(1, K * P)           # [(k p)]
    ones_kp = np.ones((1, K * P), np.float32)
    lhs = np.concatenate([-2.0 * pf, p2, ones_kp], axis=0).astype(np.float32)

    in_maps = []
    for c in range(NCORES):
        xs = x[c * NLOC:(c + 1) * NLOC]                          # (8, 8, 128)
        xf = xs.transpose(1, 0, 2).reshape(D, NLOC * W)          # [d, (n w)]
        x2 = (xs * xs).sum(axis=1).reshape(1, NLOC * W)          # [(n w)]
        ones_nw = np.ones((1, NLOC * W), np.float32)
        rhs = np.concatenate([xf, ones_nw, x2], axis=0).astype(np.float32)
        in_maps.append({"inp": np.concatenate([lhs, rhs], axis=1)})
    return in_maps


_program_cache: bass.Bass | None = None


def kernel(x: np.ndarray, patts: np.ndarray) -> np.ndarray:
    global _program_cache
    if _program_cache is None:
        _program_cache = build_program()
    nc = _program_cache
    in_maps = make_in_maps(x, patts)
    res = run_bass_kernel_spmd(nc, in_maps, list(range(NCORES)))
    return np.concatenate([r["out"] for r in res.results], axis=0)


if __name__ == "__main__":
    rng = np.random.default_rng(0)
    x = rng.standard_normal((N, D, W), dtype=np.float32)
    patts = rng.standard_normal((K, D, P), dtype=np.float32)
    out = kernel(x, patts)
    print(out.shape, out.dtype)
